# revision 14
# baseline (speedup 1.0000x reference)
"""BatchedMoE Trainium2 kernel.

Expert-parallel over 8 NeuronCores with load-balanced spill: host
computes the (tiny) router + top-2 dispatch in numpy; core c runs
expert c's gated MLP over the first 2048 tokens routed to it, one
128-token "spill" group (overflow tokens from overloaded experts,
round-robined across cores, with that expert's weights streamed
separately), plus the shared-expert MLP for a 1/8 slice of all
tokens. Matmuls run bf16 x bf16 with fp32 PSUM accumulation; the
silu/gating arithmetic stays fp32. Host scatters/combines.

Multi-tile loads use host-permuted [128, k, ...] layouts so each
logical tensor is ONE contiguous-per-partition DMA (fewer Sync-queue
dispatches, better packet aggregation).

Self-contained: only numpy + concourse imports, no sibling files.
"""
import numpy as np

B, T, C = 4, 2048, 1024
E = 8            # experts == cores
KTOP = 2         # experts per token
H = 1408         # expert intermediate
HS = 2816        # shared intermediate
N = B * T        # 8192 tokens
TPC = N // 8     # tokens per core for the shared expert
KC = C // 128    # 8 k-tiles over C
NH = H // 128    # 11 h-tiles
NHS = HS // 128  # 22 hs-tiles
NC2 = C // 512   # 2 c-halves
CAP = 2048       # main-phase capacity per core (== perfect balance)
SP = 64          # spill group width per core

TRACE = False
LAST_EXEC_NS = None
LAST_RESULTS = None

_cache = {}


def _build():
    import concourse.bacc as bacc
    import concourse.tile as tile
    import concourse.mybir as mybir
    from contextlib import ExitStack

    f32 = mybir.dt.float32
    bf16 = mybir.dt.bfloat16
    AF = mybir.ActivationFunctionType

    nc = bacc.Bacc("TRN2", target_bir_lowering=False, debug=False)

    xdT = nc.dram_tensor("xdT", [128, KC, CAP], bf16, kind="ExternalInput").ap()
    w1 = nc.dram_tensor("w1", [128, KC, H], bf16, kind="ExternalInput").ap()
    w2 = nc.dram_tensor("w2", [128, KC, H], bf16, kind="ExternalInput").ap()
    w3 = nc.dram_tensor("w3", [128, NH, C], bf16, kind="ExternalInput").ap()
    probs = nc.dram_tensor("probs", [128, CAP], f32, kind="ExternalInput").ap()
    xspT = nc.dram_tensor("xspT", [128, KC, SP], bf16, kind="ExternalInput").ap()
    wsp1 = nc.dram_tensor("wsp1", [128, KC, H], bf16, kind="ExternalInput").ap()
    wsp2 = nc.dram_tensor("wsp2", [128, KC, H], bf16, kind="ExternalInput").ap()
    wsp3 = nc.dram_tensor("wsp3", [128, NH, C], bf16, kind="ExternalInput").ap()
    probs_sp = nc.dram_tensor("probs_sp", [128, SP], f32, kind="ExternalInput").ap()
    xsT = nc.dram_tensor("xsT", [128, KC, TPC], bf16, kind="ExternalInput").ap()
    ws1b = nc.dram_tensor("ws1b", [NHS, 128, C], bf16, kind="ExternalInput").ap()
    ws2b = nc.dram_tensor("ws2b", [NHS, 128, C], bf16, kind="ExternalInput").ap()
    ws3 = nc.dram_tensor("ws3", [HS, C], bf16, kind="ExternalInput").ap()
    yd = nc.dram_tensor("yd", [CAP // 128, 128, C], bf16,
                        kind="ExternalOutput").ap()
    ysp = nc.dram_tensor("ysp", [SP, C], bf16, kind="ExternalOutput").ap()
    ys = nc.dram_tensor("ys", [TPC // 128, NC2, 128, 512], bf16,
                        kind="ExternalOutput").ap()

    NG = CAP // 512  # 4 main groups of 512
    NTH = TPC // 512
    NB_EARLY = 4  # shared-expert steps run up front as PE filler while
                  # the phase-A weights stream in

    with tile.TileContext(nc) as tc:
        with ExitStack() as outer:
            # pools that live across phases (small)
            xsp = outer.enter_context(tc.tile_pool(name="xsB", bufs=1))
            cbp = outer.enter_context(tc.tile_pool(name="cbB", bufs=4))
            hep = outer.enter_context(tc.tile_pool(name="hsE", bufs=1))
            spB = outer.enter_context(tc.tile_pool(name="tmpB", bufs=2))

            xsb = xsp.tile([128, KC, TPC], bf16, tag="xs", name="xsb")
            hstE = [hep.tile([128, TPC], bf16, tag=f"hsE{j}", name=f"hstE{j}")
                    for j in range(NB_EARLY)]

            # cb slab prefetch pipeline: DMAs for step j are emitted up to
            # two steps ahead so the A->B1 boundary is never DMA-gated.
            cbt = {}

            def issue_cb(j):
                c1 = cbp.tile([128, C], bf16, tag="cb1", name="cb1")
                nc.sync.dma_start(c1[:], ws1b[j, :, :])
                c2 = cbp.tile([128, C], bf16, tag="cb2", name="cb2")
                nc.sync.dma_start(c2[:], ws2b[j, :, :])
                cbt[j] = (c1, c2)

            def b1_step(j, hst_tile, psum_pool, amortize, ebufs=2):
                cb1, cb2 = cbt.pop(j)
                if amortize:
                    p1 = [psum_pool.tile([128, 512], f32, tag="pE1",
                                         name="pE1", bufs=3) for th in range(NTH)]
                    p2 = [psum_pool.tile([128, 512], f32, tag="pE2",
                                         name="pE2", bufs=3) for th in range(NTH)]
                    for k in range(KC):
                        for th in range(NTH):
                            nc.tensor.matmul(
                                p1[th][:], cb1[:, k * 128:(k + 1) * 128],
                                xsb[:, k, th * 512:(th + 1) * 512],
                                start=(k == 0), stop=(k == KC - 1))
                    for k in range(KC):
                        for th in range(NTH):
                            nc.tensor.matmul(
                                p2[th][:], cb2[:, k * 128:(k + 1) * 128],
                                xsb[:, k, th * 512:(th + 1) * 512],
                                start=(k == 0), stop=(k == KC - 1))
                    for th in range(NTH):
                        sl = spB.tile([128, 512], f32, tag="slB", name="slB")
                        nc.scalar.activation(sl[:], p1[th][:], AF.Silu)
                        nc.vector.tensor_mul(
                            hst_tile[:, th * 512:(th + 1) * 512],
                            sl[:], p2[th][:])
                else:
                    # low-psum variant for the early filler steps
                    for th in range(NTH):
                        p1 = psum_pool.tile([128, 512], f32, tag="pE1",
                                            name="pE1", bufs=ebufs)
                        for k in range(KC):
                            nc.tensor.matmul(
                                p1[:], cb1[:, k * 128:(k + 1) * 128],
                                xsb[:, k, th * 512:(th + 1) * 512],
                                start=(k == 0), stop=(k == KC - 1))
                        p2 = psum_pool.tile([128, 512], f32, tag="pE2",
                                            name="pE2", bufs=ebufs)
                        for k in range(KC):
                            nc.tensor.matmul(
                                p2[:], cb2[:, k * 128:(k + 1) * 128],
                                xsb[:, k, th * 512:(th + 1) * 512],
                                start=(k == 0), stop=(k == KC - 1))
                        sl = spB.tile([128, 512], f32, tag="slB", name="slB")
                        nc.scalar.activation(sl[:], p1[:], AF.Silu)
                        nc.vector.tensor_mul(
                            hst_tile[:, th * 512:(th + 1) * 512], sl[:], p2[:])

            # ---- early B1 filler: covers the phase-A weight stream ----
            # j=0 weight blocks load before xsb so the first chain starts
            # as soon as cb1 + xsb land.
            psEA = outer.enter_context(
                tc.tile_pool(name="psEA", bufs=2, space="PSUM"))
            if True:
                issue_cb(0)
                for k in range(KC):
                    nc.sync.dma_start(xsb[:, k, :], xsT[:, k, :])
                issue_cb(1)
                # Warm-up matmuls on a zeroed tile: they depend on no DMA, so
                # the PE runs them during the initial transfer instead of
                # idling, and the HAM clock gate is at 2.4 GHz (not the cold
                # 1.2) by the time real matmuls start. Results never read.
                wz = spB.tile([128, 512], bf16, tag="warmz", name="wz", bufs=1)
                nc.gpsimd.memset(wz[:], 0.0)
                pw = psEA.tile([128, 512], f32, tag="pyA", name="pw")
                for _ in range(12):
                    nc.tensor.matmul(pw[:], wz[:, :128], wz[:],
                                     start=True, stop=True)
                b1_step(0, hstE[0], psEA, amortize=False, ebufs=3)
                for j in range(1, NB_EARLY):
                    issue_cb(j + 1)
                    b1_step(j, hstE[j], psEA, amortize=False, ebufs=3)
                # j = NB_EARLY .. NB_EARLY+1 prefetched during phase A below

            # ---------------- Phase A: routed expert (main 2048) ----------------
            with ExitStack() as pa:
                wp = pa.enter_context(tc.tile_pool(name="wA", bufs=1))
                xp = pa.enter_context(tc.tile_pool(name="xA", bufs=2))
                hp = pa.enter_context(tc.tile_pool(name="hA", bufs=1))
                pp = pa.enter_context(tc.tile_pool(name="pbc", bufs=2))
                sp = pa.enter_context(tc.tile_pool(name="tmpA", bufs=2))
                op = pa.enter_context(tc.tile_pool(name="outA", bufs=2))

                w1sb = wp.tile([128, KC, H], bf16, tag="w1", name="w1sb")
                w2sb = wp.tile([128, KC, H], bf16, tag="w2", name="w2sb")
                w3sb = wp.tile([128, NH, C], bf16, tag="w3", name="w3sb")

                # group-0 x first (small), then weights in first-use order;
                # w3 trickles in behind w1/w2.
                xg = xp.tile([128, KC, 512], bf16, tag="x", name="xg")
                nc.sync.dma_start(xg[:], xdT[:, :, 0:512])
                nc.sync.dma_start(w1sb[:], w1[:, :, :])
                pb = pp.tile([128, 512], f32, tag="pb")
                nc.sync.dma_start(pb[:], probs[:, 0:512])
                nc.sync.dma_start(w2sb[:], w2[:, :, :])
                nc.sync.dma_start(w3sb[:], w3[:, :, :])

                for gi in range(NG):
                    gs = gi * 512
                    if gi > 0:
                        xg = xp.tile([128, KC, 512], bf16, tag="x", name="xg")
                        nc.sync.dma_start(xg[:], xdT[:, :, gs:gs + 512])
                        pb = pp.tile([128, 512], f32, tag="pb")
                        nc.sync.dma_start(pb[:], probs[:, gs:gs + 512])
                    if gi == NG - 1:
                        # prefetch the first two B1 steps' slabs so the
                        # A->B1 boundary is compute-gated, not DMA-gated
                        issue_cb(NB_EARLY)
                        issue_cb(NB_EARLY + 1)
                    hts = []
                    for h in range(NH):
                        p1 = psEA.tile([128, 512], f32, tag="pE1", name="pE1",
                                       bufs=3)
                        for k in range(KC):
                            nc.tensor.matmul(
                                p1[:], w1sb[:, k, h * 128:(h + 1) * 128],
                                xg[:, k, :], start=(k == 0), stop=(k == KC - 1))
                        p2 = psEA.tile([128, 512], f32, tag="pE2", name="pE2",
                                       bufs=3)
                        for k in range(KC):
                            nc.tensor.matmul(
                                p2[:], w2sb[:, k, h * 128:(h + 1) * 128],
                                xg[:, k, :], start=(k == 0), stop=(k == KC - 1))
                        sl = sp.tile([128, 512], f32, tag="sl", name="sl")
                        nc.scalar.activation(sl[:], p1[:], AF.Silu)
                        t2 = sp.tile([128, 512], f32, tag="t2", name="t2")
                        nc.vector.tensor_mul(t2[:], p2[:], pb[:])
                        ht = hp.tile([128, 512], bf16, tag=f"h{h}", name=f"ht{h}")
                        nc.vector.tensor_mul(ht[:], sl[:], t2[:])
                        hts.append(ht)

                    for t in range(4):
                        ot = op.tile([128, C], bf16, tag="ot", name="ot")
                        for c in range(NC2):
                            py = psEA.tile([128, 512], f32, tag="pyA",
                                           name="pyA")
                            for h in range(NH):
                                nc.tensor.matmul(
                                    py[:], hts[h][:, t * 128:(t + 1) * 128],
                                    w3sb[:, h, c * 512:(c + 1) * 512],
                                    start=(h == 0), stop=(h == NH - 1))
                            nc.vector.tensor_copy(
                                ot[:, c * 512:(c + 1) * 512], py[:])
                        nc.sync.dma_start(yd[gi * 4 + t, :, :], ot[:])

            # ------- Phase B1 (shared expert rest) + spill/ws3 streaming -------
            with ExitStack() as pbx:
                hbp = pbx.enter_context(tc.tile_pool(name="hsB", bufs=1))
                w3sp = pbx.enter_context(tc.tile_pool(name="ws3B", bufs=1))
                wspp = pbx.enter_context(tc.tile_pool(name="wSP", bufs=1))
                xspp = pbx.enter_context(tc.tile_pool(name="xSP", bufs=1))
                hpS = pbx.enter_context(tc.tile_pool(name="hSP", bufs=1))
                spS = pbx.enter_context(tc.tile_pool(name="tmpSP", bufs=2))
                oB = pbx.enter_context(tc.tile_pool(name="outB", bufs=2))

                hst = hstE + [
                    hbp.tile([128, TPC], bf16, tag=f"hs{j}", name=f"hst{j}")
                    for j in range(NB_EARLY, NHS)]
                ws3sb = [w3sp.tile([128, C], bf16, tag=f"ws3_{j}", name=f"ws3sb{j}")
                         for j in range(NHS)]
                wsp1sb = wspp.tile([128, KC, H], bf16, tag="sp1", name="wsp1sb")
                wsp2sb = wspp.tile([128, KC, H], bf16, tag="sp2", name="wsp2sb")
                wsp3sb = wspp.tile([128, NH, C], bf16, tag="sp3", name="wsp3sb")
                xgs = xspp.tile([128, KC, SP], bf16, tag="xsp", name="xgs")
                pbs = xspp.tile([128, SP], f32, tag="pbs", name="pbs")

                # spill-weight / spill-x / ws3 DMAs paced across the B1
                # steps in ~360KB slabs so they interleave with the cb
                # slab stream instead of blocking it.
                stream = [(xgs[:], xspT[:, :, :]), (pbs[:], probs_sp[:, :])]
                for k in range(KC):
                    stream.append((wsp1sb[:, k, :], wsp1[:, k, :]))
                for k in range(KC):
                    stream.append((wsp2sb[:, k, :], wsp2[:, k, :]))
                for h in range(NH):
                    stream.append((wsp3sb[:, h, :], wsp3[:, h, :]))
                for j in range(NHS):
                    stream.append((ws3sb[j][:], ws3[j * 128:(j + 1) * 128, :]))
                si = 0

                def pump(n):
                    nonlocal si
                    for _ in range(n):
                        if si < len(stream):
                            dst, src = stream[si]
                            nc.sync.dma_start(dst, src)
                            si += 1

                for j in range(NB_EARLY, NHS):
                    if j + 2 < NHS:
                        issue_cb(j + 2)
                    pump(3)
                    b1_step(j, hst[j], psEA, amortize=True)
                pump(len(stream))

                # ---------------- Spill: one 128-token group ----------------
                # psum reuses the (idle) B1 tags, alternating per h for
                # double-buffering; only [:, :SP] of each bank is used.
                hts_s = []
                for h in range(NH):
                    p1 = psEA.tile([128, 512], f32, tag="pE1", name="pE1",
                                   bufs=3)
                    for k in range(KC):
                        nc.tensor.matmul(
                            p1[:, :SP], wsp1sb[:, k, h * 128:(h + 1) * 128],
                            xgs[:, k, :], start=(k == 0), stop=(k == KC - 1))
                    p2 = psEA.tile([128, 512], f32, tag="pE2", name="pE2",
                                   bufs=3)
                    for k in range(KC):
                        nc.tensor.matmul(
                            p2[:, :SP], wsp2sb[:, k, h * 128:(h + 1) * 128],
                            xgs[:, k, :], start=(k == 0), stop=(k == KC - 1))
                    sl = spS.tile([128, SP], f32, tag="sp_sl", name="sp_sl")
                    nc.scalar.activation(sl[:], p1[:, :SP], AF.Silu)
                    t2 = spS.tile([128, SP], f32, tag="sp_t2", name="sp_t2")
                    nc.vector.tensor_mul(t2[:], p2[:, :SP], pbs[:])
                    ht = hpS.tile([128, SP], bf16, tag=f"sp_h{h}", name=f"hts{h}")
                    nc.vector.tensor_mul(ht[:], sl[:], t2[:])
                    hts_s.append(ht)
                for c in range(NC2):
                    pys = psEA.tile([128, 512], f32, tag="pyA", name="pyA")
                    for h in range(NH):
                        nc.tensor.matmul(
                            pys[:SP, :], hts_s[h][:],
                            wsp3sb[:, h, c * 512:(c + 1) * 512],
                            start=(h == 0), stop=(h == NH - 1))
                    ot = oB.tile([128, 512], bf16, tag="otB", name="otB")
                    nc.vector.tensor_copy(ot[:SP, :], pys[:SP, :])
                    nc.sync.dma_start(ysp[:, c * 512:(c + 1) * 512], ot[:SP, :])

                # ---------------- Phase B2: shared-expert W3 ----------------
                # c-chains serialized (not interleaved) so the final
                # copy+DMA tail after the last matmul is one tile, not two.
                for t in range(TPC // 128):
                    for c in range(NC2):
                        py = psEA.tile([128, 512], f32, tag="pyA",
                                       name="pyA")
                        for j in range(NHS):
                            nc.tensor.matmul(
                                py[:], hst[j][:, t * 128:(t + 1) * 128],
                                ws3sb[j][:, c * 512:(c + 1) * 512],
                                start=(j == 0), stop=(j == NHS - 1))
                        ot = oB.tile([128, 512], bf16, tag="otB", name="otB")
                        if t == TPC // 128 - 1 and c == NC2 - 1:
                            nc.scalar.copy(ot[:], py[:])
                        else:
                            nc.vector.tensor_copy(ot[:], py[:])
                        nc.sync.dma_start(ys[t, c, :, :], ot[:])

    nc.compile()
    return nc


def _get_nc():
    if 'v5' not in _cache:
        _cache['v5'] = _build()
    return _cache['v5']


def _kperm(a, nblk):
    """[nblk*128, F] row-major -> [128, nblk, F] with partition first."""
    f = a.shape[-1]
    return np.ascontiguousarray(
        a.reshape(nblk, 128, f).transpose(1, 0, 2))


def kernel(x, Wg, W1, W2, W3, Ws1, Ws2, Ws3):
    global LAST_EXEC_NS, LAST_RESULTS
    from concourse import bass_utils
    import ml_dtypes

    bf = ml_dtypes.bfloat16
    x = np.ascontiguousarray(np.asarray(x, dtype=np.float32))
    Wg = np.asarray(Wg, dtype=np.float32)
    W1 = np.asarray(W1, dtype=np.float32)
    W2 = np.asarray(W2, dtype=np.float32)
    W3 = np.asarray(W3, dtype=np.float32)
    Ws1 = np.asarray(Ws1, dtype=np.float32)
    Ws2 = np.asarray(Ws2, dtype=np.float32)
    Ws3 = np.asarray(Ws3, dtype=np.float32)

    xf = x.reshape(N, C)

    # ---- router + top-2 + softmax (fp32, matches jax.lax.top_k tie-break) ----
    router = xf @ Wg                                   # [N, E]
    i0 = np.argmax(router, axis=1)
    ar = np.arange(N)
    l0 = router[ar, i0]
    r2 = router.copy()
    r2[ar, i0] = -np.inf
    i1 = np.argmax(r2, axis=1)
    l1 = router[ar, i1]
    m = np.maximum(l0, l1)
    e0 = np.exp(l0 - m)
    e1 = np.exp(l1 - m)
    zs = e0 + e1
    p0 = (e0 / zs).astype(np.float32)
    p1 = (e1 / zs).astype(np.float32)

    # ---- dispatch: sort (token, slot) pairs by expert ----
    flat_e = np.concatenate([i0, i1])                  # [2N]
    flat_t = np.concatenate([ar, ar])
    flat_p = np.concatenate([p0, p1])
    order = np.argsort(flat_e, kind="stable")
    counts = np.bincount(flat_e, minlength=E)
    offs = np.zeros(E + 1, dtype=np.int64)
    np.cumsum(counts, out=offs[1:])

    # main: first CAP pairs of each expert stay on its core; the rest
    # spill in 128-wide units round-robined across cores.
    spill_units = []                # (expert, sel_indices)
    for e in range(E):
        sel = order[offs[e]:offs[e + 1]]
        for s in range(CAP, len(sel), SP):
            spill_units.append((e, sel[s:s + SP]))
    assert len(spill_units) <= E, (
        f"spill overflow: {len(spill_units)} units; counts={counts}")

    # global output slot of every pair: main pairs index into the
    # stacked [E*CAP, C] main output; spill pairs into [E*SP, C].
    gslot = np.empty(2 * N, dtype=np.int64)
    for e in range(E):
        sel = order[offs[e]:offs[e + 1]]
        nmain = min(len(sel), CAP)
        gslot[sel[:nmain]] = e * CAP + np.arange(nmain)
    for u, (e, sel) in enumerate(spill_units):
        gslot[sel] = E * CAP + u * SP + np.arange(len(sel))

    # ---- per-core inputs ----
    def blk(w, nblocks):
        return np.ascontiguousarray(
            w.reshape(KC, 128, nblocks, 128).transpose(2, 1, 0, 3)
            .reshape(nblocks, 128, C).astype(bf))

    ws1b = blk(Ws1, NHS)
    ws2b = blk(Ws2, NHS)
    ws3_bf = np.ascontiguousarray(Ws3.astype(bf))
    xfb = xf.astype(bf)
    W1b = [_kperm(W1[e].astype(bf), KC) for e in range(E)]
    W2b = [_kperm(W2[e].astype(bf), KC) for e in range(E)]
    W3b = [_kperm(W3[e].astype(bf), NH) for e in range(E)]

    in_maps = []
    for c in range(E):
        sel = order[offs[c]:offs[c + 1]][:CAP]
        toks = flat_t[sel]
        xd = np.zeros((CAP, C), dtype=bf)
        xd[:len(toks)] = xfb[toks]
        pbc = np.zeros((CAP,), dtype=np.float32)
        pbc[:len(toks)] = flat_p[sel]
        if c < len(spill_units):
            se, ssel = spill_units[c]
            stoks = flat_t[ssel]
            xsp = np.zeros((SP, C), dtype=bf)
            xsp[:len(stoks)] = xfb[stoks]
            psp = np.zeros((SP,), dtype=np.float32)
            psp[:len(stoks)] = flat_p[ssel]
            sw1, sw2, sw3 = W1b[se], W2b[se], W3b[se]
        else:
            xsp = np.zeros((SP, C), dtype=bf)
            psp = np.zeros((SP,), dtype=np.float32)
            sw1, sw2, sw3 = W1b[c], W2b[c], W3b[c]
        in_maps.append({
            "xdT": _kperm(np.ascontiguousarray(xd.T), KC),
            "w1": W1b[c],
            "w2": W2b[c],
            "w3": W3b[c],
            "probs": np.ascontiguousarray(np.broadcast_to(pbc, (128, CAP))),
            "xspT": _kperm(np.ascontiguousarray(xsp.T), KC),
            "wsp1": sw1,
            "wsp2": sw2,
            "wsp3": sw3,
            "probs_sp": np.ascontiguousarray(np.broadcast_to(psp, (128, SP))),
            "xsT": _kperm(np.ascontiguousarray(xfb[c * TPC:(c + 1) * TPC].T), KC),
            "ws1b": ws1b,
            "ws2b": ws2b,
            "ws3": ws3_bf,
        })

    nc = _get_nc()
    res = None
    for attempt in range(3):
        try:
            res = bass_utils.run_bass_kernel_spmd(
                nc, in_maps, core_ids=list(range(8)), trace=TRACE)
            break
        except Exception:
            if attempt == 2:
                raise
    LAST_EXEC_NS = res.exec_time_ns
    LAST_RESULTS = res

    # ---- combine ----
    YALL = np.concatenate(
        [np.asarray(res.results[c]["yd"]).reshape(CAP, C) for c in range(E)]
        + [np.asarray(res.results[c]["ysp"]) for c in range(E)],
        axis=0).astype(np.float32)
    y = YALL[gslot[:N]] + YALL[gslot[N:]]
    # ys comes back as [TPC//128, NC2, 128, 512] contiguous DMA blocks
    ys_all = [np.asarray(res.results[c]["ys"]).transpose(0, 2, 1, 3)
              .reshape(TPC, C) for c in range(E)]
    y += np.concatenate(ys_all, axis=0).astype(np.float32)
    return y.reshape(B, T, C)


# revision 15
# speedup vs baseline: 1.1752x; 1.1752x over previous
"""BatchedMoE Trainium2 kernel.

Expert-parallel over 8 NeuronCores with load-balanced spill: host
computes the (tiny) router + top-2 dispatch in numpy; core c runs
expert c's gated MLP over the first 2048 tokens routed to it, one
128-token "spill" group (overflow tokens from overloaded experts,
round-robined across cores, with that expert's weights streamed
separately), plus the shared-expert MLP for a 1/8 slice of all
tokens. Matmuls run bf16 x bf16 with fp32 PSUM accumulation; the
silu/gating arithmetic stays fp32. Host scatters/combines.

Multi-tile loads use host-permuted [128, k, ...] layouts so each
logical tensor is ONE contiguous-per-partition DMA (fewer Sync-queue
dispatches, better packet aggregation).

Self-contained: only numpy + concourse imports, no sibling files.
"""
import numpy as np

B, T, C = 4, 2048, 1024
E = 8            # experts == cores
KTOP = 2         # experts per token
H = 1408         # expert intermediate
HS = 2816        # shared intermediate
N = B * T        # 8192 tokens
TPC = N // 8     # tokens per core for the shared expert
KC = C // 128    # 8 k-tiles over C
NH = H // 128    # 11 h-tiles
NHS = HS // 128  # 22 hs-tiles
NC2 = C // 512   # 2 c-halves
CAP = 2048       # main-phase capacity per core (== perfect balance)
SP = 64          # spill group width per core

TRACE = False
LAST_EXEC_NS = None
LAST_RESULTS = None

_cache = {}


def _build():
    import concourse.bacc as bacc
    import concourse.tile as tile
    import concourse.mybir as mybir
    from contextlib import ExitStack

    f32 = mybir.dt.float32
    bf16 = mybir.dt.bfloat16
    AF = mybir.ActivationFunctionType

    nc = bacc.Bacc("TRN2", target_bir_lowering=False, debug=False)

    xdT = nc.dram_tensor("xdT", [128, KC, CAP], bf16, kind="ExternalInput").ap()
    w1 = nc.dram_tensor("w1", [128, KC, H], bf16, kind="ExternalInput").ap()
    w2 = nc.dram_tensor("w2", [128, KC, H], bf16, kind="ExternalInput").ap()
    w3 = nc.dram_tensor("w3", [128, NH, C], bf16, kind="ExternalInput").ap()
    probs = nc.dram_tensor("probs", [128, CAP], f32, kind="ExternalInput").ap()
    xspT = nc.dram_tensor("xspT", [128, KC, SP], bf16, kind="ExternalInput").ap()
    wsp1 = nc.dram_tensor("wsp1", [128, KC, H], bf16, kind="ExternalInput").ap()
    wsp2 = nc.dram_tensor("wsp2", [128, KC, H], bf16, kind="ExternalInput").ap()
    wsp3 = nc.dram_tensor("wsp3", [128, NH, C], bf16, kind="ExternalInput").ap()
    probs_sp = nc.dram_tensor("probs_sp", [128, SP], f32, kind="ExternalInput").ap()
    xsT = nc.dram_tensor("xsT", [128, KC, TPC], bf16, kind="ExternalInput").ap()
    ws1b = nc.dram_tensor("ws1b", [NHS, 128, C], bf16, kind="ExternalInput").ap()
    ws2b = nc.dram_tensor("ws2b", [NHS, 128, C], bf16, kind="ExternalInput").ap()
    ws3 = nc.dram_tensor("ws3", [HS, C], bf16, kind="ExternalInput").ap()
    yd = nc.dram_tensor("yd", [CAP // 128, 128, C], bf16,
                        kind="ExternalOutput").ap()
    ysp = nc.dram_tensor("ysp", [SP, C], bf16, kind="ExternalOutput").ap()
    ys = nc.dram_tensor("ys", [TPC // 128, NC2, 128, 512], bf16,
                        kind="ExternalOutput").ap()

    NG = CAP // 512  # 4 main groups of 512
    NTH = TPC // 512
    NB_EARLY = 4  # shared-expert steps run up front as PE filler while
                  # the phase-A weights stream in

    with tile.TileContext(nc) as tc:
        with ExitStack() as outer:
            # pools that live across phases (small)
            xsp = outer.enter_context(tc.tile_pool(name="xsB", bufs=1))
            cbp = outer.enter_context(tc.tile_pool(name="cbB", bufs=4))
            hep = outer.enter_context(tc.tile_pool(name="hsE", bufs=1))
            spB = outer.enter_context(tc.tile_pool(name="tmpB", bufs=2))

            xsb = xsp.tile([128, KC, TPC], bf16, tag="xs", name="xsb")
            hstE = [hep.tile([128, TPC], bf16, tag=f"hsE{j}", name=f"hstE{j}")
                    for j in range(NB_EARLY)]

            # cb slab prefetch pipeline: DMAs for step j are emitted up to
            # two steps ahead so the A->B1 boundary is never DMA-gated.
            cbt = {}

            def issue_cb(j):
                c1 = cbp.tile([128, C], bf16, tag="cb1", name="cb1")
                nc.sync.dma_start(c1[:], ws1b[j, :, :])
                c2 = cbp.tile([128, C], bf16, tag="cb2", name="cb2")
                nc.sync.dma_start(c2[:], ws2b[j, :, :])
                cbt[j] = (c1, c2)

            def b1_step(j, hst_tile, psum_pool, amortize, ebufs=2):
                cb1, cb2 = cbt.pop(j)
                if amortize:
                    p1 = [psum_pool.tile([128, 512], f32, tag="pE1",
                                         name="pE1", bufs=3) for th in range(NTH)]
                    p2 = [psum_pool.tile([128, 512], f32, tag="pE2",
                                         name="pE2", bufs=3) for th in range(NTH)]
                    for k in range(KC):
                        for th in range(NTH):
                            nc.tensor.matmul(
                                p1[th][:], cb1[:, k * 128:(k + 1) * 128],
                                xsb[:, k, th * 512:(th + 1) * 512],
                                start=(k == 0), stop=(k == KC - 1))
                    for k in range(KC):
                        for th in range(NTH):
                            nc.tensor.matmul(
                                p2[th][:], cb2[:, k * 128:(k + 1) * 128],
                                xsb[:, k, th * 512:(th + 1) * 512],
                                start=(k == 0), stop=(k == KC - 1))
                    for th in range(NTH):
                        sl = spB.tile([128, 512], f32, tag="slB", name="slB")
                        nc.scalar.activation(sl[:], p1[th][:], AF.Silu)
                        nc.vector.tensor_mul(
                            hst_tile[:, th * 512:(th + 1) * 512],
                            sl[:], p2[th][:])
                else:
                    # low-psum variant for the early filler steps
                    for th in range(NTH):
                        p1 = psum_pool.tile([128, 512], f32, tag="pE1",
                                            name="pE1", bufs=ebufs)
                        for k in range(KC):
                            nc.tensor.matmul(
                                p1[:], cb1[:, k * 128:(k + 1) * 128],
                                xsb[:, k, th * 512:(th + 1) * 512],
                                start=(k == 0), stop=(k == KC - 1))
                        p2 = psum_pool.tile([128, 512], f32, tag="pE2",
                                            name="pE2", bufs=ebufs)
                        for k in range(KC):
                            nc.tensor.matmul(
                                p2[:], cb2[:, k * 128:(k + 1) * 128],
                                xsb[:, k, th * 512:(th + 1) * 512],
                                start=(k == 0), stop=(k == KC - 1))
                        sl = spB.tile([128, 512], f32, tag="slB", name="slB")
                        nc.scalar.activation(sl[:], p1[:], AF.Silu)
                        nc.vector.tensor_mul(
                            hst_tile[:, th * 512:(th + 1) * 512], sl[:], p2[:])

            # ---- early B1 filler: covers the phase-A weight stream ----
            # j=0 weight blocks load before xsb so the first chain starts
            # as soon as cb1 + xsb land.
            psEA = outer.enter_context(
                tc.tile_pool(name="psEA", bufs=2, space="PSUM"))
            if True:
                issue_cb(0)
                for k in range(KC):
                    nc.sync.dma_start(xsb[:, k, :], xsT[:, k, :])
                issue_cb(1)
                # Warm-up matmuls on a zeroed tile: they depend on no DMA, so
                # the PE runs them during the initial transfer instead of
                # idling, and the HAM clock gate is at 2.4 GHz (not the cold
                # 1.2) by the time real matmuls start. Results never read.
                wz = spB.tile([128, 512], bf16, tag="warmz", name="wz", bufs=1)
                nc.gpsimd.memset(wz[:], 0.0)
                pw = psEA.tile([128, 512], f32, tag="pyA", name="pw")
                for _ in range(12):
                    nc.tensor.matmul(pw[:], wz[:, :128], wz[:],
                                     start=True, stop=True)
                b1_step(0, hstE[0], psEA, amortize=False, ebufs=3)
                for j in range(1, NB_EARLY):
                    issue_cb(j + 1)
                    b1_step(j, hstE[j], psEA, amortize=False, ebufs=3)
                # j = NB_EARLY .. NB_EARLY+1 prefetched during phase A below

            # ---------------- Phase A: routed expert (main 2048) ----------------
            with ExitStack() as pa:
                wp = pa.enter_context(tc.tile_pool(name="wA", bufs=1))
                xp = pa.enter_context(tc.tile_pool(name="xA", bufs=2))
                hp = pa.enter_context(tc.tile_pool(name="hA", bufs=1))
                pp = pa.enter_context(tc.tile_pool(name="pbc", bufs=2))
                sp = pa.enter_context(tc.tile_pool(name="tmpA", bufs=2))
                op = pa.enter_context(tc.tile_pool(name="outA", bufs=2))

                w1sb = wp.tile([128, KC, H], bf16, tag="w1", name="w1sb")
                w2sb = wp.tile([128, KC, H], bf16, tag="w2", name="w2sb")
                w3sb = wp.tile([128, NH, C], bf16, tag="w3", name="w3sb")

                # group-0 x first (small), then weights in first-use order;
                # w3 trickles in behind w1/w2.
                xg = xp.tile([128, KC, 512], bf16, tag="x", name="xg")
                nc.sync.dma_start(xg[:], xdT[:, :, 0:512])
                nc.sync.dma_start(w1sb[:], w1[:, :, :])
                pb = pp.tile([128, 512], f32, tag="pb")
                nc.sync.dma_start(pb[:], probs[:, 0:512])
                nc.sync.dma_start(w2sb[:], w2[:, :, :])
                nc.sync.dma_start(w3sb[:], w3[:, :, :])

                for gi in range(NG):
                    gs = gi * 512
                    if gi > 0:
                        xg = xp.tile([128, KC, 512], bf16, tag="x", name="xg")
                        nc.sync.dma_start(xg[:], xdT[:, :, gs:gs + 512])
                        pb = pp.tile([128, 512], f32, tag="pb")
                        nc.sync.dma_start(pb[:], probs[:, gs:gs + 512])
                    if gi == NG - 1:
                        # prefetch the first two B1 steps' slabs so the
                        # A->B1 boundary is compute-gated, not DMA-gated
                        issue_cb(NB_EARLY)
                        issue_cb(NB_EARLY + 1)
                    hts = []
                    for h in range(NH):
                        p1 = psEA.tile([128, 512], f32, tag="pE1", name="pE1",
                                       bufs=3)
                        for k in range(KC):
                            nc.tensor.matmul(
                                p1[:], w1sb[:, k, h * 128:(h + 1) * 128],
                                xg[:, k, :], start=(k == 0), stop=(k == KC - 1))
                        p2 = psEA.tile([128, 512], f32, tag="pE2", name="pE2",
                                       bufs=3)
                        for k in range(KC):
                            nc.tensor.matmul(
                                p2[:], w2sb[:, k, h * 128:(h + 1) * 128],
                                xg[:, k, :], start=(k == 0), stop=(k == KC - 1))
                        sl = sp.tile([128, 512], f32, tag="sl", name="sl")
                        nc.scalar.activation(sl[:], p1[:], AF.Silu)
                        t2 = sp.tile([128, 512], f32, tag="t2", name="t2")
                        nc.vector.tensor_mul(t2[:], p2[:], pb[:])
                        ht = hp.tile([128, 512], bf16, tag=f"h{h}", name=f"ht{h}")
                        nc.vector.tensor_mul(ht[:], sl[:], t2[:])
                        hts.append(ht)

                    for t in range(4):
                        ot = op.tile([128, C], bf16, tag="ot", name="ot")
                        for c in range(NC2):
                            py = psEA.tile([128, 512], f32, tag="pyA",
                                           name="pyA")
                            for h in range(NH):
                                nc.tensor.matmul(
                                    py[:], hts[h][:, t * 128:(t + 1) * 128],
                                    w3sb[:, h, c * 512:(c + 1) * 512],
                                    start=(h == 0), stop=(h == NH - 1))
                            nc.vector.tensor_copy(
                                ot[:, c * 512:(c + 1) * 512], py[:])
                        nc.sync.dma_start(yd[gi * 4 + t, :, :], ot[:])

            # ------- Phase B1 (shared expert rest) + spill/ws3 streaming -------
            with ExitStack() as pbx:
                hbp = pbx.enter_context(tc.tile_pool(name="hsB", bufs=1))
                w3sp = pbx.enter_context(tc.tile_pool(name="ws3B", bufs=1))
                wspp = pbx.enter_context(tc.tile_pool(name="wSP", bufs=1))
                xspp = pbx.enter_context(tc.tile_pool(name="xSP", bufs=1))
                hpS = pbx.enter_context(tc.tile_pool(name="hSP", bufs=1))
                spS = pbx.enter_context(tc.tile_pool(name="tmpSP", bufs=2))
                oB = pbx.enter_context(tc.tile_pool(name="outB", bufs=2))

                hst = hstE + [
                    hbp.tile([128, TPC], bf16, tag=f"hs{j}", name=f"hst{j}")
                    for j in range(NB_EARLY, NHS)]
                ws3sb = [w3sp.tile([128, C], bf16, tag=f"ws3_{j}", name=f"ws3sb{j}")
                         for j in range(NHS)]
                wsp1sb = wspp.tile([128, KC, H], bf16, tag="sp1", name="wsp1sb")
                wsp2sb = wspp.tile([128, KC, H], bf16, tag="sp2", name="wsp2sb")
                wsp3sb = wspp.tile([128, NH, C], bf16, tag="sp3", name="wsp3sb")
                xgs = xspp.tile([128, KC, SP], bf16, tag="xsp", name="xgs")
                pbs = xspp.tile([128, SP], f32, tag="pbs", name="pbs")

                # spill-weight / spill-x / ws3 DMAs paced across the B1
                # steps in ~360KB slabs so they interleave with the cb
                # slab stream instead of blocking it.
                stream = [(xgs[:], xspT[:, :, :]), (pbs[:], probs_sp[:, :])]
                for k in range(KC):
                    stream.append((wsp1sb[:, k, :], wsp1[:, k, :]))
                for k in range(KC):
                    stream.append((wsp2sb[:, k, :], wsp2[:, k, :]))
                for h in range(NH):
                    stream.append((wsp3sb[:, h, :], wsp3[:, h, :]))
                for j in range(NHS):
                    stream.append((ws3sb[j][:], ws3[j * 128:(j + 1) * 128, :]))
                si = 0

                def pump(n):
                    nonlocal si
                    for _ in range(n):
                        if si < len(stream):
                            dst, src = stream[si]
                            nc.sync.dma_start(dst, src)
                            si += 1

                for j in range(NB_EARLY, NHS):
                    if j + 2 < NHS:
                        issue_cb(j + 2)
                    pump(3)
                    b1_step(j, hst[j], psEA, amortize=True)
                pump(len(stream))

                # ---------------- Spill: one 128-token group ----------------
                # psum reuses the (idle) B1 tags, alternating per h for
                # double-buffering; only [:, :SP] of each bank is used.
                hts_s = []
                for h in range(NH):
                    p1 = psEA.tile([128, 512], f32, tag="pE1", name="pE1",
                                   bufs=3)
                    for k in range(KC):
                        nc.tensor.matmul(
                            p1[:, :SP], wsp1sb[:, k, h * 128:(h + 1) * 128],
                            xgs[:, k, :], start=(k == 0), stop=(k == KC - 1))
                    p2 = psEA.tile([128, 512], f32, tag="pE2", name="pE2",
                                   bufs=3)
                    for k in range(KC):
                        nc.tensor.matmul(
                            p2[:, :SP], wsp2sb[:, k, h * 128:(h + 1) * 128],
                            xgs[:, k, :], start=(k == 0), stop=(k == KC - 1))
                    sl = spS.tile([128, SP], f32, tag="sp_sl", name="sp_sl")
                    nc.scalar.activation(sl[:], p1[:, :SP], AF.Silu)
                    t2 = spS.tile([128, SP], f32, tag="sp_t2", name="sp_t2")
                    nc.vector.tensor_mul(t2[:], p2[:, :SP], pbs[:])
                    ht = hpS.tile([128, 128], bf16, tag=f"sp_h{h}", name=f"hts{h}")
                    nc.gpsimd.memset(ht[:, SP:], 0.0)
                    nc.vector.tensor_mul(ht[:, :SP], sl[:], t2[:])
                    hts_s.append(ht)
                for c in range(NC2):
                    pys = psEA.tile([128, 512], f32, tag="pyA", name="pyA")
                    for h in range(NH):
                        nc.tensor.matmul(
                            pys[:], hts_s[h][:],
                            wsp3sb[:, h, c * 512:(c + 1) * 512],
                            start=(h == 0), stop=(h == NH - 1))
                    ot = oB.tile([128, 512], bf16, tag="otB", name="otB")
                    nc.vector.tensor_copy(ot[:SP, :], pys[:SP, :])
                    nc.sync.dma_start(ysp[:, c * 512:(c + 1) * 512], ot[:SP, :])

                # ---------------- Phase B2: shared-expert W3 ----------------
                # c-chains serialized (not interleaved) so the final
                # copy+DMA tail after the last matmul is one tile, not two.
                for t in range(TPC // 128):
                    for c in range(NC2):
                        py = psEA.tile([128, 512], f32, tag="pyA",
                                       name="pyA")
                        for j in range(NHS):
                            nc.tensor.matmul(
                                py[:], hst[j][:, t * 128:(t + 1) * 128],
                                ws3sb[j][:, c * 512:(c + 1) * 512],
                                start=(j == 0), stop=(j == NHS - 1))
                        ot = oB.tile([128, 512], bf16, tag="otB", name="otB")
                        if t == TPC // 128 - 1 and c == NC2 - 1:
                            nc.scalar.copy(ot[:], py[:])
                        else:
                            nc.vector.tensor_copy(ot[:], py[:])
                        nc.sync.dma_start(ys[t, c, :, :], ot[:])

    nc.compile()
    return nc


def _get_nc():
    if 'v5' not in _cache:
        _cache['v5'] = _build()
    return _cache['v5']


def _kperm(a, nblk):
    """[nblk*128, F] row-major -> [128, nblk, F] with partition first."""
    f = a.shape[-1]
    return np.ascontiguousarray(
        a.reshape(nblk, 128, f).transpose(1, 0, 2))


def kernel(x, Wg, W1, W2, W3, Ws1, Ws2, Ws3):
    global LAST_EXEC_NS, LAST_RESULTS
    from concourse import bass_utils
    import ml_dtypes

    bf = ml_dtypes.bfloat16
    x = np.ascontiguousarray(np.asarray(x, dtype=np.float32))
    Wg = np.asarray(Wg, dtype=np.float32)
    W1 = np.asarray(W1, dtype=np.float32)
    W2 = np.asarray(W2, dtype=np.float32)
    W3 = np.asarray(W3, dtype=np.float32)
    Ws1 = np.asarray(Ws1, dtype=np.float32)
    Ws2 = np.asarray(Ws2, dtype=np.float32)
    Ws3 = np.asarray(Ws3, dtype=np.float32)

    xf = x.reshape(N, C)

    # ---- router + top-2 + softmax (fp32, matches jax.lax.top_k tie-break) ----
    router = xf @ Wg                                   # [N, E]
    i0 = np.argmax(router, axis=1)
    ar = np.arange(N)
    l0 = router[ar, i0]
    r2 = router.copy()
    r2[ar, i0] = -np.inf
    i1 = np.argmax(r2, axis=1)
    l1 = router[ar, i1]
    m = np.maximum(l0, l1)
    e0 = np.exp(l0 - m)
    e1 = np.exp(l1 - m)
    zs = e0 + e1
    p0 = (e0 / zs).astype(np.float32)
    p1 = (e1 / zs).astype(np.float32)

    # ---- dispatch: sort (token, slot) pairs by expert ----
    flat_e = np.concatenate([i0, i1])                  # [2N]
    flat_t = np.concatenate([ar, ar])
    flat_p = np.concatenate([p0, p1])
    order = np.argsort(flat_e, kind="stable")
    counts = np.bincount(flat_e, minlength=E)
    offs = np.zeros(E + 1, dtype=np.int64)
    np.cumsum(counts, out=offs[1:])

    # main: first CAP pairs of each expert stay on its core; the rest
    # spill in 128-wide units round-robined across cores.
    spill_units = []                # (expert, sel_indices)
    for e in range(E):
        sel = order[offs[e]:offs[e + 1]]
        for s in range(CAP, len(sel), SP):
            spill_units.append((e, sel[s:s + SP]))
    assert len(spill_units) <= E, (
        f"spill overflow: {len(spill_units)} units; counts={counts}")

    # global output slot of every pair: main pairs index into the
    # stacked [E*CAP, C] main output; spill pairs into [E*SP, C].
    gslot = np.empty(2 * N, dtype=np.int64)
    for e in range(E):
        sel = order[offs[e]:offs[e + 1]]
        nmain = min(len(sel), CAP)
        gslot[sel[:nmain]] = e * CAP + np.arange(nmain)
    for u, (e, sel) in enumerate(spill_units):
        gslot[sel] = E * CAP + u * SP + np.arange(len(sel))

    # ---- per-core inputs ----
    def blk(w, nblocks):
        return np.ascontiguousarray(
            w.reshape(KC, 128, nblocks, 128).transpose(2, 1, 0, 3)
            .reshape(nblocks, 128, C).astype(bf))

    ws1b = blk(Ws1, NHS)
    ws2b = blk(Ws2, NHS)
    ws3_bf = np.ascontiguousarray(Ws3.astype(bf))
    xfb = xf.astype(bf)
    W1b = [_kperm(W1[e].astype(bf), KC) for e in range(E)]
    W2b = [_kperm(W2[e].astype(bf), KC) for e in range(E)]
    W3b = [_kperm(W3[e].astype(bf), NH) for e in range(E)]

    in_maps = []
    for c in range(E):
        sel = order[offs[c]:offs[c + 1]][:CAP]
        toks = flat_t[sel]
        xd = np.zeros((CAP, C), dtype=bf)
        xd[:len(toks)] = xfb[toks]
        pbc = np.zeros((CAP,), dtype=np.float32)
        pbc[:len(toks)] = flat_p[sel]
        if c < len(spill_units):
            se, ssel = spill_units[c]
            stoks = flat_t[ssel]
            xsp = np.zeros((SP, C), dtype=bf)
            xsp[:len(stoks)] = xfb[stoks]
            psp = np.zeros((SP,), dtype=np.float32)
            psp[:len(stoks)] = flat_p[ssel]
            sw1, sw2, sw3 = W1b[se], W2b[se], W3b[se]
        else:
            xsp = np.zeros((SP, C), dtype=bf)
            psp = np.zeros((SP,), dtype=np.float32)
            sw1, sw2, sw3 = W1b[c], W2b[c], W3b[c]
        in_maps.append({
            "xdT": _kperm(np.ascontiguousarray(xd.T), KC),
            "w1": W1b[c],
            "w2": W2b[c],
            "w3": W3b[c],
            "probs": np.ascontiguousarray(np.broadcast_to(pbc, (128, CAP))),
            "xspT": _kperm(np.ascontiguousarray(xsp.T), KC),
            "wsp1": sw1,
            "wsp2": sw2,
            "wsp3": sw3,
            "probs_sp": np.ascontiguousarray(np.broadcast_to(psp, (128, SP))),
            "xsT": _kperm(np.ascontiguousarray(xfb[c * TPC:(c + 1) * TPC].T), KC),
            "ws1b": ws1b,
            "ws2b": ws2b,
            "ws3": ws3_bf,
        })

    nc = _get_nc()
    res = None
    for attempt in range(3):
        try:
            res = bass_utils.run_bass_kernel_spmd(
                nc, in_maps, core_ids=list(range(8)), trace=TRACE)
            break
        except Exception:
            if attempt == 2:
                raise
    LAST_EXEC_NS = res.exec_time_ns
    LAST_RESULTS = res

    # ---- combine ----
    YALL = np.concatenate(
        [np.asarray(res.results[c]["yd"]).reshape(CAP, C) for c in range(E)]
        + [np.asarray(res.results[c]["ysp"]) for c in range(E)],
        axis=0).astype(np.float32)
    y = YALL[gslot[:N]] + YALL[gslot[N:]]
    # ys comes back as [TPC//128, NC2, 128, 512] contiguous DMA blocks
    ys_all = [np.asarray(res.results[c]["ys"]).transpose(0, 2, 1, 3)
              .reshape(TPC, C) for c in range(E)]
    y += np.concatenate(ys_all, axis=0).astype(np.float32)
    return y.reshape(B, T, C)


# revision 16
# speedup vs baseline: 1.1965x; 1.0181x over previous
"""BatchedMoE Trainium2 kernel.

Expert-parallel over 8 NeuronCores with load-balanced spill: host
computes the (tiny) router + top-2 dispatch in numpy; core c runs
expert c's gated MLP over the first 2048 tokens routed to it, one
128-token "spill" group (overflow tokens from overloaded experts,
round-robined across cores, with that expert's weights streamed
separately), plus the shared-expert MLP for a 1/8 slice of all
tokens. Matmuls run bf16 x bf16 with fp32 PSUM accumulation; the
silu/gating arithmetic stays fp32. Host scatters/combines.

Multi-tile loads use host-permuted [128, k, ...] layouts so each
logical tensor is ONE contiguous-per-partition DMA (fewer Sync-queue
dispatches, better packet aggregation).

Self-contained: only numpy + concourse imports, no sibling files.
"""
import numpy as np

B, T, C = 4, 2048, 1024
E = 8            # experts == cores
KTOP = 2         # experts per token
H = 1408         # expert intermediate
HS = 2816        # shared intermediate
N = B * T        # 8192 tokens
TPC = N // 8     # tokens per core for the shared expert
KC = C // 128    # 8 k-tiles over C
NH = H // 128    # 11 h-tiles
NHS = HS // 128  # 22 hs-tiles
NC2 = C // 512   # 2 c-halves
CAP = 2048       # main-phase capacity per core (== perfect balance)
SP = 64          # spill group width per core

TRACE = False
LAST_EXEC_NS = None
LAST_RESULTS = None

_cache = {}


def _build():
    import concourse.bacc as bacc
    import concourse.tile as tile
    import concourse.mybir as mybir
    from contextlib import ExitStack

    f32 = mybir.dt.float32
    bf16 = mybir.dt.bfloat16
    AF = mybir.ActivationFunctionType

    nc = bacc.Bacc("TRN2", target_bir_lowering=False, debug=False)

    xdT = nc.dram_tensor("xdT", [128, KC, CAP], bf16, kind="ExternalInput").ap()
    w1 = nc.dram_tensor("w1", [128, KC, H], bf16, kind="ExternalInput").ap()
    w2 = nc.dram_tensor("w2", [128, KC, H], bf16, kind="ExternalInput").ap()
    w3 = nc.dram_tensor("w3", [128, NH, C], bf16, kind="ExternalInput").ap()
    probs = nc.dram_tensor("probs", [128, CAP], f32, kind="ExternalInput").ap()
    xspT = nc.dram_tensor("xspT", [128, KC, SP], bf16, kind="ExternalInput").ap()
    wsp1 = nc.dram_tensor("wsp1", [128, KC, H], bf16, kind="ExternalInput").ap()
    wsp2 = nc.dram_tensor("wsp2", [128, KC, H], bf16, kind="ExternalInput").ap()
    wsp3 = nc.dram_tensor("wsp3", [128, NH, C], bf16, kind="ExternalInput").ap()
    probs_sp = nc.dram_tensor("probs_sp", [128, SP], f32, kind="ExternalInput").ap()
    xsT = nc.dram_tensor("xsT", [128, KC, TPC], bf16, kind="ExternalInput").ap()
    ws1b = nc.dram_tensor("ws1b", [NHS, 128, C], bf16, kind="ExternalInput").ap()
    ws2b = nc.dram_tensor("ws2b", [NHS, 128, C], bf16, kind="ExternalInput").ap()
    ws3 = nc.dram_tensor("ws3", [HS, C], bf16, kind="ExternalInput").ap()
    yd = nc.dram_tensor("yd", [CAP // 128, 128, C], bf16,
                        kind="ExternalOutput").ap()
    ysp = nc.dram_tensor("ysp", [SP, C], bf16, kind="ExternalOutput").ap()
    ys = nc.dram_tensor("ys", [TPC // 128, NC2, 128, 512], bf16,
                        kind="ExternalOutput").ap()

    NG = CAP // 512  # 4 main groups of 512
    NTH = TPC // 512
    NB_EARLY = 5  # shared-expert steps run up front as PE filler while
                  # the phase-A weights stream in

    with tile.TileContext(nc) as tc:
        with ExitStack() as outer:
            # pools that live across phases (small)
            xsp = outer.enter_context(tc.tile_pool(name="xsB", bufs=1))
            cbp = outer.enter_context(tc.tile_pool(name="cbB", bufs=4))
            hep = outer.enter_context(tc.tile_pool(name="hsE", bufs=1))
            spB = outer.enter_context(tc.tile_pool(name="tmpB", bufs=2))

            xsb = xsp.tile([128, KC, TPC], bf16, tag="xs", name="xsb")
            hstE = [hep.tile([128, TPC], bf16, tag=f"hsE{j}", name=f"hstE{j}")
                    for j in range(NB_EARLY)]

            # cb slab prefetch pipeline: DMAs for step j are emitted up to
            # two steps ahead so the A->B1 boundary is never DMA-gated.
            cbt = {}

            def issue_cb(j):
                c1 = cbp.tile([128, C], bf16, tag="cb1", name="cb1")
                nc.sync.dma_start(c1[:], ws1b[j, :, :])
                c2 = cbp.tile([128, C], bf16, tag="cb2", name="cb2")
                nc.sync.dma_start(c2[:], ws2b[j, :, :])
                cbt[j] = (c1, c2)

            def b1_step(j, hst_tile, psum_pool, amortize, ebufs=2):
                cb1, cb2 = cbt.pop(j)
                if amortize:
                    p1 = [psum_pool.tile([128, 512], f32, tag="pE1",
                                         name="pE1", bufs=3) for th in range(NTH)]
                    p2 = [psum_pool.tile([128, 512], f32, tag="pE2",
                                         name="pE2", bufs=3) for th in range(NTH)]
                    for k in range(KC):
                        for th in range(NTH):
                            nc.tensor.matmul(
                                p1[th][:], cb1[:, k * 128:(k + 1) * 128],
                                xsb[:, k, th * 512:(th + 1) * 512],
                                start=(k == 0), stop=(k == KC - 1))
                    for k in range(KC):
                        for th in range(NTH):
                            nc.tensor.matmul(
                                p2[th][:], cb2[:, k * 128:(k + 1) * 128],
                                xsb[:, k, th * 512:(th + 1) * 512],
                                start=(k == 0), stop=(k == KC - 1))
                    for th in range(NTH):
                        sl = spB.tile([128, 512], f32, tag="slB", name="slB")
                        nc.scalar.activation(sl[:], p1[th][:], AF.Silu)
                        nc.vector.tensor_mul(
                            hst_tile[:, th * 512:(th + 1) * 512],
                            sl[:], p2[th][:])
                else:
                    # low-psum variant for the early filler steps
                    for th in range(NTH):
                        p1 = psum_pool.tile([128, 512], f32, tag="pE1",
                                            name="pE1", bufs=ebufs)
                        for k in range(KC):
                            nc.tensor.matmul(
                                p1[:], cb1[:, k * 128:(k + 1) * 128],
                                xsb[:, k, th * 512:(th + 1) * 512],
                                start=(k == 0), stop=(k == KC - 1))
                        p2 = psum_pool.tile([128, 512], f32, tag="pE2",
                                            name="pE2", bufs=ebufs)
                        for k in range(KC):
                            nc.tensor.matmul(
                                p2[:], cb2[:, k * 128:(k + 1) * 128],
                                xsb[:, k, th * 512:(th + 1) * 512],
                                start=(k == 0), stop=(k == KC - 1))
                        sl = spB.tile([128, 512], f32, tag="slB", name="slB")
                        nc.scalar.activation(sl[:], p1[:], AF.Silu)
                        nc.vector.tensor_mul(
                            hst_tile[:, th * 512:(th + 1) * 512], sl[:], p2[:])

            # ---- early B1 filler: covers the phase-A weight stream ----
            # j=0 weight blocks load before xsb so the first chain starts
            # as soon as cb1 + xsb land.
            psEA = outer.enter_context(
                tc.tile_pool(name="psEA", bufs=2, space="PSUM"))
            if True:
                issue_cb(0)
                for k in range(KC):
                    nc.sync.dma_start(xsb[:, k, :], xsT[:, k, :])
                issue_cb(1)
                # Warm-up matmuls on a zeroed tile: they depend on no DMA, so
                # the PE runs them during the initial transfer instead of
                # idling, and the HAM clock gate is at 2.4 GHz (not the cold
                # 1.2) by the time real matmuls start. Results never read.
                wz = spB.tile([128, 512], bf16, tag="warmz", name="wz", bufs=1)
                nc.gpsimd.memset(wz[:], 0.0)
                pw = psEA.tile([128, 512], f32, tag="pyA", name="pw")
                for _ in range(12):
                    nc.tensor.matmul(pw[:], wz[:, :128], wz[:],
                                     start=True, stop=True)
                b1_step(0, hstE[0], psEA, amortize=False, ebufs=3)
                for j in range(1, NB_EARLY):
                    issue_cb(j + 1)
                    b1_step(j, hstE[j], psEA, amortize=False, ebufs=3)
                # j = NB_EARLY .. NB_EARLY+1 prefetched during phase A below

            # ---------------- Phase A: routed expert (main 2048) ----------------
            with ExitStack() as pa:
                wp = pa.enter_context(tc.tile_pool(name="wA", bufs=1))
                xp = pa.enter_context(tc.tile_pool(name="xA", bufs=2))
                hp = pa.enter_context(tc.tile_pool(name="hA", bufs=1))
                pp = pa.enter_context(tc.tile_pool(name="pbc", bufs=2))
                sp = pa.enter_context(tc.tile_pool(name="tmpA", bufs=2))
                op = pa.enter_context(tc.tile_pool(name="outA", bufs=2))

                w1sb = wp.tile([128, KC, H], bf16, tag="w1", name="w1sb")
                w2sb = wp.tile([128, KC, H], bf16, tag="w2", name="w2sb")
                w3sb = wp.tile([128, NH, C], bf16, tag="w3", name="w3sb")

                # group-0 x first (small), then weights in first-use order;
                # w3 trickles in behind w1/w2.
                xg = xp.tile([128, KC, 512], bf16, tag="x", name="xg")
                nc.sync.dma_start(xg[:], xdT[:, :, 0:512])
                nc.sync.dma_start(w1sb[:], w1[:, :, :])
                pb = pp.tile([128, 512], f32, tag="pb")
                nc.sync.dma_start(pb[:], probs[:, 0:512])
                nc.sync.dma_start(w2sb[:], w2[:, :, :])
                nc.sync.dma_start(w3sb[:], w3[:, :, :])

                for gi in range(NG):
                    gs = gi * 512
                    if gi > 0:
                        xg = xp.tile([128, KC, 512], bf16, tag="x", name="xg")
                        nc.sync.dma_start(xg[:], xdT[:, :, gs:gs + 512])
                        pb = pp.tile([128, 512], f32, tag="pb")
                        nc.sync.dma_start(pb[:], probs[:, gs:gs + 512])
                    if gi == NG - 1:
                        # prefetch the first two B1 steps' slabs so the
                        # A->B1 boundary is compute-gated, not DMA-gated
                        issue_cb(NB_EARLY)
                        issue_cb(NB_EARLY + 1)
                    hts = []
                    for h in range(NH):
                        p1 = psEA.tile([128, 512], f32, tag="pE1", name="pE1",
                                       bufs=3)
                        for k in range(KC):
                            nc.tensor.matmul(
                                p1[:], w1sb[:, k, h * 128:(h + 1) * 128],
                                xg[:, k, :], start=(k == 0), stop=(k == KC - 1))
                        p2 = psEA.tile([128, 512], f32, tag="pE2", name="pE2",
                                       bufs=3)
                        for k in range(KC):
                            nc.tensor.matmul(
                                p2[:], w2sb[:, k, h * 128:(h + 1) * 128],
                                xg[:, k, :], start=(k == 0), stop=(k == KC - 1))
                        sl = sp.tile([128, 512], f32, tag="sl", name="sl")
                        nc.scalar.activation(sl[:], p1[:], AF.Silu)
                        t2 = sp.tile([128, 512], f32, tag="t2", name="t2")
                        nc.vector.tensor_mul(t2[:], p2[:], pb[:])
                        ht = hp.tile([128, 512], bf16, tag=f"h{h}", name=f"ht{h}")
                        nc.vector.tensor_mul(ht[:], sl[:], t2[:])
                        hts.append(ht)

                    for t in range(4):
                        ot = op.tile([128, C], bf16, tag="ot", name="ot")
                        for c in range(NC2):
                            py = psEA.tile([128, 512], f32, tag="pyA",
                                           name="pyA")
                            for h in range(NH):
                                nc.tensor.matmul(
                                    py[:], hts[h][:, t * 128:(t + 1) * 128],
                                    w3sb[:, h, c * 512:(c + 1) * 512],
                                    start=(h == 0), stop=(h == NH - 1))
                            nc.vector.tensor_copy(
                                ot[:, c * 512:(c + 1) * 512], py[:])
                        nc.sync.dma_start(yd[gi * 4 + t, :, :], ot[:])

            # ------- Phase B1 (shared expert rest) + spill/ws3 streaming -------
            with ExitStack() as pbx:
                hbp = pbx.enter_context(tc.tile_pool(name="hsB", bufs=1))
                w3sp = pbx.enter_context(tc.tile_pool(name="ws3B", bufs=1))
                wspp = pbx.enter_context(tc.tile_pool(name="wSP", bufs=1))
                xspp = pbx.enter_context(tc.tile_pool(name="xSP", bufs=1))
                hpS = pbx.enter_context(tc.tile_pool(name="hSP", bufs=1))
                spS = pbx.enter_context(tc.tile_pool(name="tmpSP", bufs=2))
                oB = pbx.enter_context(tc.tile_pool(name="outB", bufs=2))

                hst = hstE + [
                    hbp.tile([128, TPC], bf16, tag=f"hs{j}", name=f"hst{j}")
                    for j in range(NB_EARLY, NHS)]
                ws3sb = [w3sp.tile([128, C], bf16, tag=f"ws3_{j}", name=f"ws3sb{j}")
                         for j in range(NHS)]
                wsp1sb = wspp.tile([128, KC, H], bf16, tag="sp1", name="wsp1sb")
                wsp2sb = wspp.tile([128, KC, H], bf16, tag="sp2", name="wsp2sb")
                wsp3sb = wspp.tile([128, NH, C], bf16, tag="sp3", name="wsp3sb")
                xgs = xspp.tile([128, KC, SP], bf16, tag="xsp", name="xgs")
                pbs = xspp.tile([128, SP], f32, tag="pbs", name="pbs")

                # spill-weight / spill-x / ws3 DMAs paced across the B1
                # steps in ~360KB slabs so they interleave with the cb
                # slab stream instead of blocking it.
                stream = [(xgs[:], xspT[:, :, :]), (pbs[:], probs_sp[:, :])]
                for k in range(KC):
                    stream.append((wsp1sb[:, k, :], wsp1[:, k, :]))
                for k in range(KC):
                    stream.append((wsp2sb[:, k, :], wsp2[:, k, :]))
                for h in range(NH):
                    stream.append((wsp3sb[:, h, :], wsp3[:, h, :]))
                for j in range(NHS):
                    stream.append((ws3sb[j][:], ws3[j * 128:(j + 1) * 128, :]))
                si = 0

                def pump(n):
                    nonlocal si
                    for _ in range(n):
                        if si < len(stream):
                            dst, src = stream[si]
                            nc.sync.dma_start(dst, src)
                            si += 1

                for j in range(NB_EARLY, NHS):
                    if j + 2 < NHS:
                        issue_cb(j + 2)
                    pump(3)
                    b1_step(j, hst[j], psEA, amortize=True)
                pump(len(stream))

                # ---------------- Spill: one 128-token group ----------------
                # psum reuses the (idle) B1 tags, alternating per h for
                # double-buffering; only [:, :SP] of each bank is used.
                hts_s = []
                for h in range(NH):
                    p1 = psEA.tile([128, 512], f32, tag="pE1", name="pE1",
                                   bufs=3)
                    for k in range(KC):
                        nc.tensor.matmul(
                            p1[:, :SP], wsp1sb[:, k, h * 128:(h + 1) * 128],
                            xgs[:, k, :], start=(k == 0), stop=(k == KC - 1))
                    p2 = psEA.tile([128, 512], f32, tag="pE2", name="pE2",
                                   bufs=3)
                    for k in range(KC):
                        nc.tensor.matmul(
                            p2[:, :SP], wsp2sb[:, k, h * 128:(h + 1) * 128],
                            xgs[:, k, :], start=(k == 0), stop=(k == KC - 1))
                    sl = spS.tile([128, SP], f32, tag="sp_sl", name="sp_sl")
                    nc.scalar.activation(sl[:], p1[:, :SP], AF.Silu)
                    t2 = spS.tile([128, SP], f32, tag="sp_t2", name="sp_t2")
                    nc.vector.tensor_mul(t2[:], p2[:, :SP], pbs[:])
                    ht = hpS.tile([128, 128], bf16, tag=f"sp_h{h}", name=f"hts{h}")
                    nc.gpsimd.memset(ht[:, SP:], 0.0)
                    nc.vector.tensor_mul(ht[:, :SP], sl[:], t2[:])
                    hts_s.append(ht)
                for c in range(NC2):
                    pys = psEA.tile([128, 512], f32, tag="pyA", name="pyA")
                    for h in range(NH):
                        nc.tensor.matmul(
                            pys[:], hts_s[h][:],
                            wsp3sb[:, h, c * 512:(c + 1) * 512],
                            start=(h == 0), stop=(h == NH - 1))
                    ot = oB.tile([128, 512], bf16, tag="otB", name="otB")
                    nc.vector.tensor_copy(ot[:SP, :], pys[:SP, :])
                    nc.sync.dma_start(ysp[:, c * 512:(c + 1) * 512], ot[:SP, :])

                # ---------------- Phase B2: shared-expert W3 ----------------
                # c-chains serialized (not interleaved) so the final
                # copy+DMA tail after the last matmul is one tile, not two.
                for t in range(TPC // 128):
                    for c in range(NC2):
                        py = psEA.tile([128, 512], f32, tag="pyA",
                                       name="pyA")
                        for j in range(NHS):
                            nc.tensor.matmul(
                                py[:], hst[j][:, t * 128:(t + 1) * 128],
                                ws3sb[j][:, c * 512:(c + 1) * 512],
                                start=(j == 0), stop=(j == NHS - 1))
                        ot = oB.tile([128, 512], bf16, tag="otB", name="otB")
                        if t == TPC // 128 - 1 and c == NC2 - 1:
                            nc.scalar.copy(ot[:], py[:])
                        else:
                            nc.vector.tensor_copy(ot[:], py[:])
                        nc.sync.dma_start(ys[t, c, :, :], ot[:])

    nc.compile()
    return nc


def _get_nc():
    if 'v5' not in _cache:
        _cache['v5'] = _build()
    return _cache['v5']


def _kperm(a, nblk):
    """[nblk*128, F] row-major -> [128, nblk, F] with partition first."""
    f = a.shape[-1]
    return np.ascontiguousarray(
        a.reshape(nblk, 128, f).transpose(1, 0, 2))


def kernel(x, Wg, W1, W2, W3, Ws1, Ws2, Ws3):
    global LAST_EXEC_NS, LAST_RESULTS
    from concourse import bass_utils
    import ml_dtypes

    bf = ml_dtypes.bfloat16
    x = np.ascontiguousarray(np.asarray(x, dtype=np.float32))
    Wg = np.asarray(Wg, dtype=np.float32)
    W1 = np.asarray(W1, dtype=np.float32)
    W2 = np.asarray(W2, dtype=np.float32)
    W3 = np.asarray(W3, dtype=np.float32)
    Ws1 = np.asarray(Ws1, dtype=np.float32)
    Ws2 = np.asarray(Ws2, dtype=np.float32)
    Ws3 = np.asarray(Ws3, dtype=np.float32)

    xf = x.reshape(N, C)

    # ---- router + top-2 + softmax (fp32, matches jax.lax.top_k tie-break) ----
    router = xf @ Wg                                   # [N, E]
    i0 = np.argmax(router, axis=1)
    ar = np.arange(N)
    l0 = router[ar, i0]
    r2 = router.copy()
    r2[ar, i0] = -np.inf
    i1 = np.argmax(r2, axis=1)
    l1 = router[ar, i1]
    m = np.maximum(l0, l1)
    e0 = np.exp(l0 - m)
    e1 = np.exp(l1 - m)
    zs = e0 + e1
    p0 = (e0 / zs).astype(np.float32)
    p1 = (e1 / zs).astype(np.float32)

    # ---- dispatch: sort (token, slot) pairs by expert ----
    flat_e = np.concatenate([i0, i1])                  # [2N]
    flat_t = np.concatenate([ar, ar])
    flat_p = np.concatenate([p0, p1])
    order = np.argsort(flat_e, kind="stable")
    counts = np.bincount(flat_e, minlength=E)
    offs = np.zeros(E + 1, dtype=np.int64)
    np.cumsum(counts, out=offs[1:])

    # main: first CAP pairs of each expert stay on its core; the rest
    # spill in 128-wide units round-robined across cores.
    spill_units = []                # (expert, sel_indices)
    for e in range(E):
        sel = order[offs[e]:offs[e + 1]]
        for s in range(CAP, len(sel), SP):
            spill_units.append((e, sel[s:s + SP]))
    assert len(spill_units) <= E, (
        f"spill overflow: {len(spill_units)} units; counts={counts}")

    # global output slot of every pair: main pairs index into the
    # stacked [E*CAP, C] main output; spill pairs into [E*SP, C].
    gslot = np.empty(2 * N, dtype=np.int64)
    for e in range(E):
        sel = order[offs[e]:offs[e + 1]]
        nmain = min(len(sel), CAP)
        gslot[sel[:nmain]] = e * CAP + np.arange(nmain)
    for u, (e, sel) in enumerate(spill_units):
        gslot[sel] = E * CAP + u * SP + np.arange(len(sel))

    # ---- per-core inputs ----
    def blk(w, nblocks):
        return np.ascontiguousarray(
            w.reshape(KC, 128, nblocks, 128).transpose(2, 1, 0, 3)
            .reshape(nblocks, 128, C).astype(bf))

    ws1b = blk(Ws1, NHS)
    ws2b = blk(Ws2, NHS)
    ws3_bf = np.ascontiguousarray(Ws3.astype(bf))
    xfb = xf.astype(bf)
    W1b = [_kperm(W1[e].astype(bf), KC) for e in range(E)]
    W2b = [_kperm(W2[e].astype(bf), KC) for e in range(E)]
    W3b = [_kperm(W3[e].astype(bf), NH) for e in range(E)]

    in_maps = []
    for c in range(E):
        sel = order[offs[c]:offs[c + 1]][:CAP]
        toks = flat_t[sel]
        xd = np.zeros((CAP, C), dtype=bf)
        xd[:len(toks)] = xfb[toks]
        pbc = np.zeros((CAP,), dtype=np.float32)
        pbc[:len(toks)] = flat_p[sel]
        if c < len(spill_units):
            se, ssel = spill_units[c]
            stoks = flat_t[ssel]
            xsp = np.zeros((SP, C), dtype=bf)
            xsp[:len(stoks)] = xfb[stoks]
            psp = np.zeros((SP,), dtype=np.float32)
            psp[:len(stoks)] = flat_p[ssel]
            sw1, sw2, sw3 = W1b[se], W2b[se], W3b[se]
        else:
            xsp = np.zeros((SP, C), dtype=bf)
            psp = np.zeros((SP,), dtype=np.float32)
            sw1, sw2, sw3 = W1b[c], W2b[c], W3b[c]
        in_maps.append({
            "xdT": _kperm(np.ascontiguousarray(xd.T), KC),
            "w1": W1b[c],
            "w2": W2b[c],
            "w3": W3b[c],
            "probs": np.ascontiguousarray(np.broadcast_to(pbc, (128, CAP))),
            "xspT": _kperm(np.ascontiguousarray(xsp.T), KC),
            "wsp1": sw1,
            "wsp2": sw2,
            "wsp3": sw3,
            "probs_sp": np.ascontiguousarray(np.broadcast_to(psp, (128, SP))),
            "xsT": _kperm(np.ascontiguousarray(xfb[c * TPC:(c + 1) * TPC].T), KC),
            "ws1b": ws1b,
            "ws2b": ws2b,
            "ws3": ws3_bf,
        })

    nc = _get_nc()
    res = None
    for attempt in range(3):
        try:
            res = bass_utils.run_bass_kernel_spmd(
                nc, in_maps, core_ids=list(range(8)), trace=TRACE)
            break
        except Exception:
            if attempt == 2:
                raise
    LAST_EXEC_NS = res.exec_time_ns
    LAST_RESULTS = res

    # ---- combine ----
    YALL = np.concatenate(
        [np.asarray(res.results[c]["yd"]).reshape(CAP, C) for c in range(E)]
        + [np.asarray(res.results[c]["ysp"]) for c in range(E)],
        axis=0).astype(np.float32)
    y = YALL[gslot[:N]] + YALL[gslot[N:]]
    # ys comes back as [TPC//128, NC2, 128, 512] contiguous DMA blocks
    ys_all = [np.asarray(res.results[c]["ys"]).transpose(0, 2, 1, 3)
              .reshape(TPC, C) for c in range(E)]
    y += np.concatenate(ys_all, axis=0).astype(np.float32)
    return y.reshape(B, T, C)
